# revision 1
# baseline (speedup 1.0000x reference)
"""Multi-head attention (B=4, S=2048, D=1024, H=16) on 8 TRN2 NeuronCores.

Sharding: 2D grid (batch x head-group). Core c = g*4 + b handles batch b and
head group g (8 heads = 512 of the 1024 embedding columns).

Per-core kernel (all matmul operands float32r: fp32-grade data processed at
1 cycle/row by the PE; every operand tile is written by a compute engine so
it is rounded to FP32r as the BIR verifier requires):
  1. x_b^T [1024, 2048] (host pre-transposes) DMA'd in chunks, rounded to
     f32r, resident in SBUF.
  2. Per head-pair p (4 pairs of 2 heads = 128 cols): Q^T/K^T/V^T
     [128, 2048] = W_pair^T @ x^T (PSUM accumulation over 8 k-chunks, bias
     added on PSUM->SBUF eviction). V^T is PE-transposed into V "normal"
     layout per head with a ones column appended, so the P@V matmul also
     emits the softmax denominator as its last output row.
  3. Attention per head in scores-transposed layout: S^T [k-tile 128,
     q 1024] = K^T_tile^T @ Q^T (head slices live at partition base 0/64,
     which the PE accepts). exp on ScalarE with scale=1/8 folded in; no
     max-subtraction (scores are ~N(0,1) by construction, exp is safe).
     attn^T[65, 512] += [V_h | 1]^T @ expS over all 16 k-tiles. Row 64 =
     sumexp -> reciprocal -> K=1 ones-matmul broadcasts it across 64
     partitions -> multiply normalizes attn^T.
  4. Partial output projection out_part [2048, 1024] = attn_c @ Wo[cols_g].
Host sums the two head-group partials per batch and adds bo.
"""
import numpy as np

B, S, D, H, DH = 4, 2048, 1024, 16, 64
NCORES = 8
GCOLS = D // 2          # 512 cols per head-group core
NPAIRS = GCOLS // 128   # 4 head-pairs per core
NKT = S // 128          # 16 k-tiles
NQT = S // 1024         # 2 q-tile-pairs of 1024
DC = D // 128           # 8 contraction chunks for projections

_COMPILED = None


def _build():
    import concourse.bass as bass
    import concourse.bacc as bacc
    import concourse.tile as tile
    from concourse import mybir
    from concourse.masks import make_identity
    from contextlib import ExitStack

    F32 = mybir.dt.float32
    F32R = mybir.dt.float32r
    EXP = mybir.ActivationFunctionType.Exp

    nc = bacc.Bacc("TRN2", target_bir_lowering=False, debug=False)
    xT = nc.dram_tensor("xT", [D, S], F32, kind="ExternalInput").ap()
    wq = nc.dram_tensor("wq", [D, GCOLS], F32, kind="ExternalInput").ap()
    wk = nc.dram_tensor("wk", [D, GCOLS], F32, kind="ExternalInput").ap()
    wv = nc.dram_tensor("wv", [D, GCOLS], F32, kind="ExternalInput").ap()
    wo = nc.dram_tensor("wo", [GCOLS, D], F32, kind="ExternalInput").ap()
    bq = nc.dram_tensor("bq", [GCOLS], F32, kind="ExternalInput").ap()
    bk = nc.dram_tensor("bk", [GCOLS], F32, kind="ExternalInput").ap()
    bv = nc.dram_tensor("bv", [GCOLS], F32, kind="ExternalInput").ap()
    out = nc.dram_tensor("out", [S, D], F32, kind="ExternalOutput").ap()

    with tile.TileContext(nc) as tc, ExitStack() as outer:
        const = outer.enter_context(tc.tile_pool(name="const", bufs=1))
        persist = outer.enter_context(tc.tile_pool(name="persist", bufs=1))

        idf = const.tile([128, 128], F32)
        make_identity(nc, idf)
        idr = const.tile([128, 128], F32R)
        nc.vector.tensor_copy(idr, idf)
        ones_f = const.tile([128, 64], F32)
        nc.vector.memset(ones_f, 1.0)
        bq_sb = const.tile([128, NPAIRS], F32)
        bk_sb = const.tile([128, NPAIRS], F32)
        bv_sb = const.tile([128, NPAIRS], F32)
        nc.sync.dma_start(out=bq_sb, in_=bq.rearrange("(p r) -> r p", r=128))
        nc.sync.dma_start(out=bk_sb, in_=bk.rearrange("(p r) -> r p", r=128))
        nc.sync.dma_start(out=bv_sb, in_=bv.rearrange("(p r) -> r p", r=128))

        # x^T resident, rounded to f32r via DVE copy (verifier requirement)
        xT_r = persist.tile([128, DC, S], F32R)
        xT_dram = xT.rearrange("(dc p) n -> p dc n", p=128)

        attnT = [persist.tile([128, S], F32R, name=f"attnT{p}", tag=f"attnT{p}")
                 for p in range(NPAIRS)]

        with ExitStack() as inner:
            xstage = inner.enter_context(tc.tile_pool(name="xstage", bufs=2))
            wstage = inner.enter_context(tc.tile_pool(name="wstage", bufs=1))
            wpool = inner.enter_context(tc.tile_pool(name="wpool", bufs=1))
            qkv = inner.enter_context(tc.tile_pool(name="qkv", bufs=1))
            vpool = inner.enter_context(tc.tile_pool(name="vpool", bufs=1))
            espool = inner.enter_context(tc.tile_pool(name="espool", bufs=5))
            small = inner.enter_context(tc.tile_pool(name="small", bufs=2))
            ps512 = inner.enter_context(
                tc.tile_pool(name="ps512", bufs=2, space="PSUM"))
            pssc = inner.enter_context(
                tc.tile_pool(name="pssc", bufs=2, space="PSUM"))
            psav = inner.enter_context(
                tc.tile_pool(name="psav", bufs=2, space="PSUM"))

            zf = xstage.tile([128, 512], F32, name="zf", tag="zf")
            nc.vector.memset(zf, 0.0)
            zr = xstage.tile([128, 512], F32R, name="zr", tag="zr")
            nc.vector.tensor_copy(zr, zf)
            warm_ps = ps512.tile([128, 512], F32, name="warm_ps",
                                 tag="ps512")
            for _ in range(40):
                nc.tensor.matmul(warm_ps, idr, zr, start=True, stop=True,
                                 skip_group_check=True)

            qeng = [nc.sync, nc.scalar]
            for half in range(4):
                for dc in range(DC):
                    xs = xstage.tile([128, S // 4], F32, name="xs", tag="xs")
                    cols = slice(half * (S // 4), (half + 1) * (S // 4))
                    qeng[(half * DC + dc) % 2].dma_start(
                        out=xs, in_=xT_dram[:, dc, cols])
                    nc.vector.tensor_copy(xT_r[:, dc, cols], xs)

            for p in range(NPAIRS):
                csl = slice(p * 128, (p + 1) * 128)
                # --- projections: Q^T/K^T/V^T pair tiles [128, S]
                pair_t = {}
                for nm, w_ap, b_sb in (("q", wq, bq_sb), ("k", wk, bk_sb),
                                       ("v", wv, bv_sb)):
                    w_r = wpool.tile([128, DC, 128], F32R, name=f"w{nm}_r",
                                     tag=f"w{nm}")
                    wre = w_ap.rearrange("(dc p) m -> p dc m", p=128)
                    for wh in range(2):
                        ws = wstage.tile([128, DC // 2, 128], F32, name="ws",
                                         tag="ws", bufs=2)
                        dsl = slice(wh * (DC // 2), (wh + 1) * (DC // 2))
                        nc.sync.dma_start(out=ws, in_=wre[:, dsl, csl])
                        nc.vector.tensor_copy(w_r[:, dsl, :], ws)
                    t_sb = qkv.tile([128, S], F32R, name=f"{nm}t_sb",
                                    tag=f"{nm}t",
                                    bufs=(1 if nm == "v" else 2))
                    for nt in range(S // 512):
                        mm_ps = ps512.tile([128, 512], F32, name="proj_ps",
                                           tag="ps512")
                        for dc in range(DC):
                            nc.tensor.matmul(
                                mm_ps, w_r[:, dc, :],
                                xT_r[:, dc, nt * 512:(nt + 1) * 512],
                                start=(dc == 0), stop=(dc == DC - 1))
                        nc.scalar.activation(
                            t_sb[:, nt * 512:(nt + 1) * 512], mm_ps,
                            mybir.ActivationFunctionType.Identity,
                            bias=b_sb[:, p:p + 1])
                    pair_t[nm] = t_sb
                qt_sb, kt_sb, vt_sb = pair_t["q"], pair_t["k"], pair_t["v"]

                # --- V^T -> V normal layout [k, 65] per head (ones col last)
                v_sb = vpool.tile([128, NKT, 130], F32R)
                ones3 = ones_f.rearrange("p (a b) -> p a b", b=1)[:, 0:NKT, :]
                nc.vector.tensor_copy(v_sb[:, :, 64:65], ones3)
                nc.vector.tensor_copy(v_sb[:, :, 129:130], ones3)
                for kb in range(NKT):
                    tr_ps = ps512.tile([128, 128], F32R, name="tr_ps",
                                       tag="ps512")
                    nc.tensor.matmul(tr_ps, vt_sb[:, kb * 128:(kb + 1) * 128],
                                     idr, is_transpose=True,
                                     start=True, stop=True)
                    nc.scalar.activation(v_sb[:, kb, 0:64],
                                         tr_ps[:, 0:64],
                                         mybir.ActivationFunctionType.Copy)
                    nc.scalar.activation(v_sb[:, kb, 65:129],
                                         tr_ps[:, 64:128],
                                         mybir.ActivationFunctionType.Copy)

                # --- attention per head
                for hh in range(2):
                    hb = hh * 64
                    vw = slice(hh * 65, hh * 65 + 65)
                    for qt in range(NQT):
                        q0 = qt * 1024
                        av_ps = [psav.tile([65, 512], F32, name=f"av_ps{qh}",
                                           tag="psav") for qh in range(2)]
                        for kt in range(NKT):
                            sc_ps = pssc.tile([128, 1024], F32, name="sc_ps",
                                              tag="pssc")
                            for qh in range(2):
                                nc.tensor.matmul(
                                    sc_ps[:, qh * 512:(qh + 1) * 512],
                                    kt_sb[hb:hb + 64,
                                          kt * 128:(kt + 1) * 128],
                                    qt_sb[hb:hb + 64,
                                          q0 + qh * 512:q0 + (qh + 1) * 512],
                                    start=True, stop=True)
                            es = espool.tile([128, 1024], F32R, name="es",
                                             tag="es")
                            nc.scalar.activation(es, sc_ps, EXP, scale=0.125)
                            for qh in range(2):
                                nc.tensor.matmul(
                                    av_ps[qh], v_sb[:, kt, vw],
                                    es[:, qh * 512:(qh + 1) * 512],
                                    start=(kt == 0), stop=(kt == NKT - 1),
                                    skip_group_check=True)
                        for qh in range(2):
                            col = slice(q0 + qh * 512, q0 + (qh + 1) * 512)
                            av_sb = small.tile([65, 512], F32,
                                               name="av_sb", tag="av_sb")
                            nc.vector.tensor_copy(av_sb, av_ps[qh])
                            bc = small.tile([64, 512], F32, name="bc",
                                            tag="bc", bufs=1)
                            sr = av_sb[64:65, :]
                            rep = bass.AP(tensor=sr.tensor, offset=sr.offset,
                                          ap=[sr.ap[0], [0, 64], [1, 512]])
                            nc.sync.dma_start(out=bc.unsqueeze(1), in_=rep)
                            rec = small.tile([64, 512], F32, name="rec",
                                             tag="rec")
                            nc.vector.reciprocal_approx_fast(out=rec, in_=bc)
                            if hh == 0:
                                nc.vector.tensor_mul(attnT[p][0:64, col],
                                                     av_sb[0:64, :], rec)
                            else:
                                tmp = small.tile([64, 512], F32R, name="tmp",
                                                 tag="tmp", bufs=1)
                                nc.vector.tensor_mul(tmp, av_sb[0:64, :],
                                                     rec)
                                nc.sync.dma_start(out=attnT[p][64:128, col],
                                                  in_=tmp)

        # --- output projection: out[q, :] = sum_p attnT[p]^T @ wo rows
        with ExitStack() as fin:
            wostage = fin.enter_context(tc.tile_pool(name="wostage", bufs=1))
            wopool = fin.enter_context(tc.tile_pool(name="wopool", bufs=1))
            osb = fin.enter_context(tc.tile_pool(name="osb", bufs=4))
            psout = fin.enter_context(
                tc.tile_pool(name="psout", bufs=4, space="PSUM"))
            wo_st = wostage.tile([128, NPAIRS, D], F32)
            nc.sync.dma_start(out=wo_st,
                              in_=wo.rearrange("(p r) n -> r p n", r=128))
            wo_r = wopool.tile([128, NPAIRS, D], F32R)
            nc.vector.tensor_copy(wo_r, wo_st)
            for qc in range(S // 128):
                o_ps = [psout.tile([128, 512], F32, name=f"o_ps{nt}",
                                   tag="psout") for nt in range(2)]
                for p in range(NPAIRS):
                    for nt in range(2):
                        nc.tensor.matmul(
                            o_ps[nt],
                            attnT[p][:, qc * 128:(qc + 1) * 128],
                            wo_r[:, p, nt * 512:(nt + 1) * 512],
                            start=(p == 0), stop=(p == NPAIRS - 1),
                            skip_group_check=True)
                for nt in range(2):
                    o_sb = osb.tile([128, 512], F32, name="o_sb", tag="o_sb")
                    nc.vector.tensor_copy(o_sb, o_ps[nt])
                    nc.sync.dma_start(
                        out=out[qc * 128:(qc + 1) * 128,
                                nt * 512:(nt + 1) * 512],
                        in_=o_sb)

    nc.compile()
    return nc


def _get_compiled():
    global _COMPILED
    if _COMPILED is None:
        _COMPILED = _build()
    return _COMPILED


def make_in_maps(**inputs):
    x = np.asarray(inputs["inputs"], np.float32)
    xTb = [np.ascontiguousarray(x[b].T) for b in range(B)]
    gslice = {}
    for nm in ("Wq", "Wk", "Wv", "Wo", "bq", "bk", "bv"):
        a = np.asarray(inputs[nm], np.float32)
        for g in range(2):
            sl = slice(g * GCOLS, (g + 1) * GCOLS)
            if nm == "Wo":
                gslice[(nm, g)] = np.ascontiguousarray(a[sl, :])
            elif nm.startswith("W"):
                gslice[(nm, g)] = np.ascontiguousarray(a[:, sl])
            else:
                gslice[(nm, g)] = np.ascontiguousarray(a[sl])
    in_maps = []
    for c in range(NCORES):
        g, b = c // B, c % B
        in_maps.append({
            "xT": xTb[b],
            "wq": gslice[("Wq", g)], "wk": gslice[("Wk", g)],
            "wv": gslice[("Wv", g)], "wo": gslice[("Wo", g)],
            "bq": gslice[("bq", g)], "bk": gslice[("bk", g)],
            "bv": gslice[("bv", g)],
        })
    return in_maps


def combine(results, bo):
    out = np.empty((B, S, D), np.float32)
    bo = np.asarray(bo, np.float32)
    for b in range(B):
        out[b] = results[b]["out"] + results[B + b]["out"] + bo
    return out


def kernel(**inputs):
    from concourse import bass_utils
    nc = _get_compiled()
    in_maps = make_in_maps(**inputs)
    res = bass_utils.run_bass_kernel_spmd(
        nc, in_maps, core_ids=list(range(NCORES)))
    return combine(res.results, inputs["bo"])



# revision 4
# speedup vs baseline: 1.4289x; 1.4289x over previous
"""Multi-head attention (B=4, S=2048, D=1024, H=16) on 8 TRN2 NeuronCores.

Sharding: 2D grid (batch x head-group). Core c = g*4 + b handles batch b and
head group g (8 heads = 512 of the 1024 embedding columns).

v2 kernel: all matmul operands bf16 (host pre-casts x^T and weights, so no
on-chip rounding passes); fp32 PSUM accumulation everywhere.

Per-core phases:
  1. x^T [1024, 2048] bf16 DMA'd per k-tile; V = x @ Wv computed directly in
     normal [k, d] layout (no PE transposes), evicted with bias into
     vones[128, kt, 8*65] bf16 where each head slot is [V_h | 1]. Then
     K^T / Q^T pair-0 tiles [128, 2048] bf16 (head 2p at rows 0-63, 2p+1 at
     64-127); Q scaled by 1/8 at eviction so exp needs no scale.
  2. Attention per pair, per q-chunk of 512: score slices (kt, head) stream
     through PSUM tiles of 2 resp. 3 slices ([128, {2,3}, 512] f32); the two
     heads' score matmuls (contraction 64, partition bases 0/64) run
     concurrently in the PE via row tiling. One EXP per tile (N=1024/1536)
     evicts to bf16 es. PV matmuls ([V_h|1] stationary, es moving) run one
     tile behind the scores (software pipelining, so the PE never waits on
     exp) and accumulate av[65, 512] per head over all 16 kt (row 64 =
     sumexp). Normalize via DMA-broadcast of sumexp + DVE reciprocal*mul
     into attnT bf16. Next pair's Q^T/K^T projection matmuls (and, during
     the last pair, output-projection chunks) are dribbled between tiles
     into the PE's exp-wait slack using the 1 spare PSUM bank.
  3. Remaining output projection out = sum_p attnT[p]^T @ Wo[p] at the end.
Host sums the two head-group partials per batch and adds bo.
"""
import numpy as np

B, S, D, H, DH = 4, 2048, 1024, 16, 64
NCORES = 8
GCOLS = D // 2          # 512 cols per head-group core
NPAIRS = GCOLS // 128   # 4 head-pairs per core
NKT = S // 128          # 16 k-tiles
DC = D // 8 // 16       # 8 contraction chunks of 128 for projections
DC = 8
NQQ = 4                 # q processed in 512-wide chunks
GROUPS = [2, 3] * 6 + [2]   # 32 (kt, head) slices per (pair, qq)

_COMPILED = None


def _build():
    import concourse.bass as bass
    import concourse.bacc as bacc
    import concourse.tile as tile
    from concourse import mybir
    from contextlib import ExitStack

    F32 = mybir.dt.float32
    BF16 = mybir.dt.bfloat16
    EXP = mybir.ActivationFunctionType.Exp
    ADD = mybir.AluOpType.add
    MULT = mybir.AluOpType.mult

    nc = bacc.Bacc("TRN2", target_bir_lowering=False, debug=False)
    xT = nc.dram_tensor("xT", [D, S], BF16, kind="ExternalInput").ap()
    wq = nc.dram_tensor("wq", [D, GCOLS], BF16, kind="ExternalInput").ap()
    wk = nc.dram_tensor("wk", [D, GCOLS], BF16, kind="ExternalInput").ap()
    wv = nc.dram_tensor("wv", [D, GCOLS], BF16, kind="ExternalInput").ap()
    wo = nc.dram_tensor("wo", [GCOLS, D], BF16, kind="ExternalInput").ap()
    bq = nc.dram_tensor("bq", [GCOLS], F32, kind="ExternalInput").ap()
    bk = nc.dram_tensor("bk", [GCOLS], F32, kind="ExternalInput").ap()
    bv = nc.dram_tensor("bv", [GCOLS], F32, kind="ExternalInput").ap()
    out = nc.dram_tensor("out", [S, D], F32, kind="ExternalOutput").ap()

    with tile.TileContext(nc) as tc, ExitStack() as outer:
        const = outer.enter_context(tc.tile_pool(name="const", bufs=1))
        persist = outer.enter_context(tc.tile_pool(name="persist", bufs=1))

        # --- resident inputs (bf16, DMA'd directly, no staging) ---
        xT_sb = persist.tile([128, DC, S], BF16)
        xT_dram = xT.rearrange("(dc p) n -> p dc n", p=128)
        # per-kt DMAs so V matmuls start as soon as each k-tile arrives
        for kt in range(NKT):
            ksl = slice(kt * 128, (kt + 1) * 128)
            nc.sync.dma_start(out=xT_sb[:, :, ksl], in_=xT_dram[:, :, ksl])

        wq_sb = persist.tile([128, DC, GCOLS], BF16)
        wk_sb = persist.tile([128, DC, GCOLS], BF16)
        wv_sb = persist.tile([128, DC, GCOLS], BF16)
        wo_sb = persist.tile([128, NPAIRS, D], BF16)
        nc.scalar.dma_start(out=wv_sb,
                            in_=wv.rearrange("(dc p) m -> p dc m", p=128))
        nc.scalar.dma_start(out=wk_sb,
                            in_=wk.rearrange("(dc p) m -> p dc m", p=128))
        nc.scalar.dma_start(out=wq_sb,
                            in_=wq.rearrange("(dc p) m -> p dc m", p=128))
        nc.scalar.dma_start(out=wo_sb,
                            in_=wo.rearrange("(p r) n -> r p n", r=128))

        bq_sb = const.tile([128, NPAIRS], F32)
        bk_sb = const.tile([128, NPAIRS], F32)
        nc.scalar.dma_start(out=bq_sb, in_=bq.rearrange("(p r) -> r p", r=128))
        nc.scalar.dma_start(out=bk_sb, in_=bk.rearrange("(p r) -> r p", r=128))
        # bv broadcast-replicated across partitions: [128, 512]
        bv_bc = const.tile([128, GCOLS], F32)
        bv_rep = bass.AP(tensor=bv.tensor, offset=bv.offset,
                         ap=[[0, 128], [1, GCOLS]])
        nc.scalar.dma_start(out=bv_bc, in_=bv_rep)

        # V in normal layout with ones col per head: [128, kt, 8*65]
        vones = persist.tile([128, NKT, 8 * 65], BF16)
        v4 = vones.rearrange("p k (h c) -> p k h c", c=65)
        for kt in range(NKT):
            nc.vector.memset(v4[:, kt, :, 64:65], 1.0)

        qt_sb = [persist.tile([128, S], BF16, name=f"qt{p}", tag=f"qt{p}")
                 for p in range(NPAIRS)]
        kt_sb = [persist.tile([128, S], BF16, name=f"kt{p}", tag=f"kt{p}")
                 for p in range(NPAIRS)]
        attnT = [persist.tile([128, S], BF16, name=f"attnT{p}",
                              tag=f"attnT{p}") for p in range(NPAIRS)]

        def emit_qk_chunk(pool, p, nm, w_sb, b_sb, dst, nt):
            """One 512-token chunk of a Q^T/K^T pair projection (eager)."""
            csl = slice(p * 128, (p + 1) * 128)
            nsl = slice(nt * 512, (nt + 1) * 512)
            ps = pool.tile([128, 512], F32, name="proj_ps", tag="proj")
            for dc in range(DC):
                nc.tensor.matmul(ps, w_sb[:, dc, csl], xT_sb[:, dc, nsl],
                                 start=(dc == 0), stop=(dc == DC - 1))
            if nm == "q":  # fold the 1/sqrt(dh)=1/8 softmax scale into Q
                nc.vector.tensor_scalar(out=dst[:, nsl], in0=ps,
                                        scalar1=b_sb[:, p:p + 1],
                                        scalar2=0.125, op0=ADD, op1=MULT)
            else:
                nc.vector.tensor_scalar(out=dst[:, nsl], in0=ps,
                                        scalar1=b_sb[:, p:p + 1],
                                        scalar2=None, op0=ADD)

        # ---------------- phase 1: V (all heads) + pair-0 K^T/Q^T ----------
        with ExitStack() as ph1:
            pwide = ph1.enter_context(
                tc.tile_pool(name="pwide", bufs=2, space="PSUM"))
            projps1 = ph1.enter_context(
                tc.tile_pool(name="projps1", bufs=2, space="PSUM"))

            # V = x @ Wv per 2-kt group: psum [128, 2, 512]
            bv_h = bv_bc.rearrange("p (h c) -> p h c", c=64)
            for kg in range(NKT // 2):
                pv = pwide.tile([128, 2, GCOLS], F32, name="pv", tag="pw")
                for j in range(2):
                    kt = kg * 2 + j
                    ksl = slice(kt * 128, (kt + 1) * 128)
                    for dc in range(DC):
                        nc.tensor.matmul(
                            pv[:, j, :], xT_sb[:, dc, ksl], wv_sb[:, dc, :],
                            start=(dc == 0), stop=(dc == DC - 1),
                            skip_group_check=True)
                pv_h = pv.rearrange("p j (h c) -> p j h c", c=64)
                for j in range(2):
                    kt = kg * 2 + j
                    nc.vector.tensor_tensor(
                        out=v4[:, kt, :, 0:64], in0=pv_h[:, j, :, :],
                        in1=bv_h, op=ADD)

            for nt in range(4):
                emit_qk_chunk(projps1, 0, "k", wk_sb, bk_sb, kt_sb[0], nt)
            for nt in range(4):
                emit_qk_chunk(projps1, 0, "q", wq_sb, bq_sb, qt_sb[0], nt)

        # ---------------- phase 2: attention (+ dribbled projections) ------
        with ExitStack() as ph2:
            scpool = ph2.enter_context(
                tc.tile_pool(name="scpool", bufs=1, space="PSUM"))
            avpool = ph2.enter_context(
                tc.tile_pool(name="avpool", bufs=1, space="PSUM"))
            projps2 = ph2.enter_context(
                tc.tile_pool(name="projps2", bufs=1, space="PSUM"))
            espool = ph2.enter_context(tc.tile_pool(name="espool", bufs=2))
            small = ph2.enter_context(tc.tile_pool(name="small", bufs=2))
            osb = ph2.enter_context(tc.tile_pool(name="osb", bufs=2))

            # --- dribble queue: closures emitting one instruction each ---
            dq = []

            def queue_qk_chunk(p, nm, w_sb, b_sb, dst, nt):
                csl = slice(p * 128, (p + 1) * 128)
                nsl = slice(nt * 512, (nt + 1) * 512)
                cell = {}

                def mk_mm(dc):
                    def f():
                        if dc == 0:
                            cell["ps"] = projps2.tile([128, 512], F32,
                                                      name="proj_ps",
                                                      tag="proj")
                        nc.tensor.matmul(cell["ps"], w_sb[:, dc, csl],
                                         xT_sb[:, dc, nsl],
                                         start=(dc == 0), stop=(dc == DC - 1),
                                         skip_group_check=True)
                    return f

                def mk_evict():
                    def f():
                        if nm == "q":
                            nc.vector.tensor_scalar(
                                out=dst[:, nsl], in0=cell["ps"],
                                scalar1=b_sb[:, p:p + 1],
                                scalar2=0.125, op0=ADD, op1=MULT)
                        else:
                            nc.vector.tensor_scalar(
                                out=dst[:, nsl], in0=cell["ps"],
                                scalar1=b_sb[:, p:p + 1],
                                scalar2=None, op0=ADD)
                    return f

                for dc in range(DC):
                    dq.append(mk_mm(dc))
                dq.append(mk_evict())

            def queue_out_chunk(qc):
                """Output projection for one 128-row q chunk."""
                for nt in range(2):
                    cell = {}

                    def mk_mm(p, nt=nt):
                        def f():
                            if p == 0:
                                cell["ps"] = projps2.tile([128, 512], F32,
                                                          name="o_ps",
                                                          tag="proj")
                            nc.tensor.matmul(
                                cell["ps"],
                                attnT[p][:, qc * 128:(qc + 1) * 128],
                                wo_sb[:, p, nt * 512:(nt + 1) * 512],
                                start=(p == 0), stop=(p == NPAIRS - 1),
                                skip_group_check=True)
                        return f

                    def mk_evict(nt=nt):
                        def f():
                            o_sb = osb.tile([128, 512], F32, name="o_sb",
                                            tag="o_sb")
                            nc.vector.tensor_copy(o_sb, cell["ps"])
                            nc.sync.dma_start(
                                out=out[qc * 128:(qc + 1) * 128,
                                        nt * 512:(nt + 1) * 512],
                                in_=o_sb)
                        return f

                    for p in range(NPAIRS):
                        dq.append(mk_mm(p))
                    dq.append(mk_evict())

            done_out = 0  # out-proj chunks queued so far

            for p in range(NPAIRS):
                for qq in range(NQQ):
                    qsl = slice(qq * 512, (qq + 1) * 512)
                    # refill dribble queue at qq boundaries
                    if p + 1 < NPAIRS:
                        queue_qk_chunk(p + 1, "k", wk_sb, bk_sb,
                                       kt_sb[p + 1], qq)
                        queue_qk_chunk(p + 1, "q", wq_sb, bq_sb,
                                       qt_sb[p + 1], qq)
                    elif qq > 0:
                        # last pair: dribble out-proj of finished q ranges
                        for qc in range(done_out, qq * 4):
                            queue_out_chunk(qc)
                            done_out = qq * 4

                    av = [avpool.tile([65, 512], F32, name=f"av{h}",
                                      tag=f"av{h}") for h in range(2)]
                    prev = None  # software-pipelined PV (one tile behind)
                    s = 0
                    for gsz in GROUPS:
                        scq = scpool.tile([128, gsz, 512], F32, name="scq",
                                          tag=f"scq{gsz}")
                        es = espool.tile([128, gsz, 512], BF16, name="es",
                                         tag=f"es{gsz}")
                        sl = [((s + j) >> 1, (s + j) & 1) for j in range(gsz)]
                        for j, (kt, hd) in enumerate(sl):
                            rows = slice(hd * 64, hd * 64 + 64)
                            nc.tensor.matmul(
                                scq[:, j, :],
                                kt_sb[p][rows, kt * 128:(kt + 1) * 128],
                                qt_sb[p][rows, qsl],
                                start=True, stop=True)
                        if prev is not None:
                            pes, psl = prev
                            for j, (kt, hd) in enumerate(psl):
                                slot = 2 * p + hd
                                nc.tensor.matmul(
                                    av[hd],
                                    vones[:, kt, slot * 65:slot * 65 + 65],
                                    pes[:, j, :],
                                    start=(kt == 0), stop=(kt == NKT - 1),
                                    skip_group_check=True)
                        nc.scalar.activation(es, scq, EXP)
                        # fill PE slack with queued projection work
                        for _ in range(2 if p + 1 < NPAIRS else 3):
                            if dq:
                                dq.pop(0)()
                        prev = (es, sl)
                        s += gsz
                    pes, psl = prev
                    for j, (kt, hd) in enumerate(psl):
                        slot = 2 * p + hd
                        nc.tensor.matmul(
                            av[hd], vones[:, kt, slot * 65:slot * 65 + 65],
                            pes[:, j, :],
                            start=(kt == 0), stop=(kt == NKT - 1),
                            skip_group_check=True)

                    # normalize + write attnT (bf16)
                    for hd in range(2):
                        av_sb = small.tile([65, 512], F32, name="av_sb",
                                           tag="av_sb")
                        nc.vector.tensor_copy(av_sb, av[hd])
                        bc = small.tile([64, 512], F32, name="bc", tag="bc")
                        sr = av_sb[64:65, :]
                        rep = bass.AP(tensor=sr.tensor, offset=sr.offset,
                                      ap=[sr.ap[0], [0, 64], [1, 512]])
                        nc.sync.dma_start(out=bc.unsqueeze(1), in_=rep)
                        rec = small.tile([64, 512], F32, name="rec",
                                         tag="rec")
                        nc.vector.reciprocal_approx_fast(out=rec, in_=bc)
                        if hd == 0:
                            nc.vector.tensor_mul(attnT[p][0:64, qsl],
                                                 av_sb[0:64, :], rec)
                        else:
                            tmp = small.tile([64, 512], BF16, name="tmp",
                                             tag="tmp", bufs=1)
                            nc.vector.tensor_mul(tmp, av_sb[0:64, :], rec)
                            nc.sync.dma_start(out=attnT[p][64:128, qsl],
                                              in_=tmp)

            # drain any leftover dribble ops (incl. final out-proj chunks)
            for qc in range(done_out, S // 128):
                queue_out_chunk(qc)
            while dq:
                dq.pop(0)()

    nc.compile()
    return nc


def _get_compiled():
    global _COMPILED
    if _COMPILED is None:
        _COMPILED = _build()
    return _COMPILED


def make_in_maps(**inputs):
    import ml_dtypes
    bf16 = ml_dtypes.bfloat16
    x = np.asarray(inputs["inputs"], np.float32)
    xTb = [np.ascontiguousarray(x[b].T).astype(bf16) for b in range(B)]
    gslice = {}
    for nm in ("Wq", "Wk", "Wv", "Wo", "bq", "bk", "bv"):
        a = np.asarray(inputs[nm], np.float32)
        for g in range(2):
            sl = slice(g * GCOLS, (g + 1) * GCOLS)
            if nm == "Wo":
                gslice[(nm, g)] = np.ascontiguousarray(a[sl, :]).astype(bf16)
            elif nm.startswith("W"):
                gslice[(nm, g)] = np.ascontiguousarray(a[:, sl]).astype(bf16)
            else:
                gslice[(nm, g)] = np.ascontiguousarray(a[sl])
    in_maps = []
    for c in range(NCORES):
        g, b = c // B, c % B
        in_maps.append({
            "xT": xTb[b],
            "wq": gslice[("Wq", g)], "wk": gslice[("Wk", g)],
            "wv": gslice[("Wv", g)], "wo": gslice[("Wo", g)],
            "bq": gslice[("bq", g)], "bk": gslice[("bk", g)],
            "bv": gslice[("bv", g)],
        })
    return in_maps


def combine(results, bo):
    out = np.empty((B, S, D), np.float32)
    bo = np.asarray(bo, np.float32)
    for b in range(B):
        out[b] = results[b]["out"] + results[B + b]["out"] + bo
    return out


def kernel(**inputs):
    from concourse import bass_utils
    nc = _get_compiled()
    in_maps = make_in_maps(**inputs)
    res = bass_utils.run_bass_kernel_spmd(
        nc, in_maps, core_ids=list(range(NCORES)))
    return combine(res.results, inputs["bo"])
